# revision 29
# baseline (speedup 1.0000x reference)
"""Trainium2 Bass kernel for nn_AttentionBlock (b=1, c=1024, l=2048, 16 heads).

Sharding: 2 heads per core across 8 cores. Each core:
  - loads full x (fp8, 16-way-split DMA), computes GroupNorm stats paced by
    the DMA, folds the affine into fp8 qkv weights (DoubleRow pairs),
  - computes its 2 heads' q/k (bf16) and v (fp8) via fp8-DoubleRow matmuls
    over kt-pairs,
  - runs fused attention transposed (attT[k,q]) in four query quarters.
    QK writes i-tile PAIRS into [128,1024] 2-bank PSUM tiles; the exact
    rel-pos bias rides a 256-wide diagonal strip deposit (tb2 table,
    identity matmul); outside the strip the saturated bias constant is
    folded into the exp as an additive bias (ACT bias operand / Schraudolph
    scalar2), so softmax numerators and the ones-column denominators stay
    consistent with zero extra work,
  - exp outputs fp8 directly: ACT Exp->fp8 or DVE int8-Schraudolph, one
    [128,1024] instruction per (pair, head) when no region boundary,
  - AV runs fp8 DoubleRow over i-tile pairs (lhsT = [128,2,128] v-weights
    with a ones column for the denominator),
  - softmax 1/den via the custom-DVE reciprocal_approx_fast (18-bit, one
    op; avoids both the 8x-slow iterative DVE reciprocal and any ACT
    table switch — the only ACT functions used anywhere are
    Exp/Ln/Square/Identity/Copy, all in natural_log_exp_and_others),
  - output projection bf16 on stacked normalized heads; partials stream
    out per quarter in [128,1024] pair-copies overlapped with attention.
Host sums the 8 partials and adds b_proj and the residual x.
"""

import math
import os
import numpy as np

N_HEAD = 16
NUM_BUCKETS = 32
MAX_DISTANCE = 64
GN_GROUPS = 32
GN_EPS = 1e-5

B, C, L = 1, 1024, 2048
DH = C // N_HEAD              # 64
N_CORES = 8
LT = L // 128                 # 16 l-tiles
CT = C // 128                 # 8 channel tiles
TBW2 = 768                    # strip bias table width (incl. saturated tail)
SCALE = 1.0 / math.sqrt(math.sqrt(DH))

# Schraudolph exp -> fp8e4m3 bits via int8: round(a*x + b), a = 8/ln2
SCHR_A8 = 8.0 / math.log(2.0)
SCHR_B8 = 7.0 * 8.0 - 0.344

EACT8 = int(os.environ.get("EACT8", "5"))     # n of 8 exp groups on ACT

_CACHE = {}


def _bucket_np(rel):
    # faithful numpy port of the reference _relative_position_bucket
    n = -rel
    nb = NUM_BUCKETS // 2
    ret = (n < 0).astype(np.int32) * nb
    n = np.abs(n)
    max_exact = nb // 2
    is_small = n < max_exact
    val_if_large = max_exact + (
        np.log(np.maximum(n, 1).astype(np.float32) / max_exact)
        / np.float32(math.log(MAX_DISTANCE / max_exact))
        * (nb - max_exact)
    ).astype(np.int32)
    val_if_large = np.minimum(val_if_large, nb - 1)
    return ret + np.where(is_small, n, val_if_large)


def _build_nc():
    import concourse.bacc as bacc
    import concourse.tile as tile
    from concourse import mybir

    F32 = mybir.dt.float32
    BF16 = mybir.dt.bfloat16
    I8 = mybir.dt.int8
    F8 = mybir.dt.float8e4
    AF = mybir.ActivationFunctionType
    ALU = mybir.AluOpType
    DR = mybir.MatmulPerfMode.DoubleRow

    nc = bacc.Bacc("TRN2", target_bir_lowering=False, debug=False,
                   num_devices=N_CORES)

    d_x = nc.dram_tensor("x", [C, L], F8, kind="ExternalInput")
    d_wqkvT = nc.dram_tensor("wqkvT", [C, 384], BF16, kind="ExternalInput")
    # consts cols: 0:4 ind, 4:12 gnw, 12:20 gnb, 20:23 bvec(q,k,v),
    # 23:27 ACT exp bias [C31_0, C31_1, C15_0, C15_1],
    # 27:31 DVE schr scalar2 for same regions
    d_consts = nc.dram_tensor("consts", [128, 31], F32, kind="ExternalInput")
    d_wproj2 = nc.dram_tensor("wproj2", [128, C], BF16, kind="ExternalInput")
    d_tb2 = nc.dram_tensor("tb2", [2, 128, TBW2], BF16, kind="ExternalInput")
    d_identb = nc.dram_tensor("identb", [128, 128], BF16, kind="ExternalInput")
    d_indT = nc.dram_tensor("indT", [4, 128], F32, kind="ExternalInput")
    d_out = nc.dram_tensor("pout", [C, L], BF16, kind="ExternalOutput")

    with tile.TileContext(nc) as tc:
        with tc.tile_pool(name="big", bufs=1) as big, \
             tc.tile_pool(name="small", bufs=1) as small:

            # ---- persistent SBUF tiles
            t_xb = big.tile([128, CT, L], F8)
            t_wqkvT = big.tile([128, CT, 384], BF16)
            t_wqkvS = big.tile([128, CT, 384], F8)
            t_consts = small.tile([128, 31], F32)
            t_wproj2 = big.tile([128, C], BF16)
            t_tb2 = small.tile([128, 2, TBW2], BF16)
            t_identb = small.tile([128, 128], BF16)
            t_indT = small.tile([4, 128], F32)
            t_eps = small.tile([128, 1], F32)
            t_q2 = big.tile([128, L], BF16)
            t_k2z0 = big.tile([128, L], BF16)
            t_k2z1 = big.tile([128, L], BF16)
            t_v2 = big.tile([128, L], BF16)
            t_outh = big.tile([128, L], BF16)
            # AV weights, fp8, i-tiles stacked for DoubleRow pairs:
            # cols 0:64 head0 v, col 64 ones (den0);
            # cols 128+64:256 head1 v, col 128+32 ones (den1)
            t_vt = big.tile([128, LT, 256], F8)
            t_dn = small.tile([128, 1024], BF16)
            t_selbc = small.tile([128, 128], BF16)
            t_bcp = [small.tile([128, 2, 512], F32, name=f"t_bc{i}")
                     for i in range(2)]

            # ---- DMAs: small tensors FIRST so identb lands before the
            # 2MB x stream hogs the DMA engines (the PE warm chain needs
            # identb at ~1us, else qkv runs at cold half-clock), then x
            # 16-way split round-robin on 3 engine queues.
            nc.sync.dma_start(out=t_identb[:], in_=d_identb[:])
            nc.sync.dma_start(out=t_consts[:], in_=d_consts[:])
            nc.sync.dma_start(out=t_indT[:], in_=d_indT[:])
            xr = d_x[:].rearrange("(t p) l -> p t l", p=128)
            qs = [nc.sync, nc.gpsimd, nc.scalar]
            for t in range(CT):
                for ph in range(2):
                    psl = slice(64 * ph, 64 * (ph + 1))
                    qs[(2 * t + ph) % 3].dma_start(out=t_xb[psl, t, :],
                                                   in_=xr[psl, t, :])
            nc.vector.memset(t_eps[:], GN_EPS)
            t_scr1 = small.tile([1, 1], F32)
            # ACT table preload for Exp/Ln, before any scalar-queue DMA work
            nc.scalar.activation(out=t_scr1[:], in_=t_eps[0:1, :],
                                 func=AF.Exp)
            wr = d_wqkvT[:].rearrange("(t p) m -> p t m", p=128)
            for tp in range(4):
                nc.scalar.dma_start(out=t_wqkvT[:, 2 * tp:2 * tp + 2, :],
                                    in_=wr[:, 2 * tp:2 * tp + 2, :])
            nc.scalar.dma_start(out=t_wproj2[:], in_=d_wproj2[:])
            nc.scalar.dma_start(out=t_tb2[:],
                                in_=d_tb2[:].rearrange("j p m -> p j m"))
            t_ind = t_consts[:, 0:4]
            t_gnw = t_consts[:, 4:12]
            t_gnb = t_consts[:, 12:20]
            t_bvec = t_consts[:, 20:23]
            # zero AV weights + set denominator ones-columns; selbc rows for
            # the denominator broadcast; all on gpsimd (DVE stays free)
            nc.gpsimd.memset(t_vt[:], 0.0)
            nc.gpsimd.memset(t_vt[:, :, 64:65], 1.0)
            nc.gpsimd.memset(t_vt[:, :, 160:161], 1.0)
            nc.gpsimd.memset(t_k2z0[64:128, :], 0.0)
            nc.gpsimd.memset(t_k2z1[0:64, :], 0.0)
            nc.gpsimd.memset(t_dn[:], 0.0)
            nc.gpsimd.memset(t_selbc[:], 0.0)
            nc.gpsimd.memset(t_selbc[32:33, :], 1.0)
            nc.gpsimd.memset(t_selbc[64:65, :], 1.0)

            # ---- GroupNorm stats paced by x DMA + PE warmup dummies.
            with tc.tile_pool(name="warm_ps", bufs=1, space="PSUM") as warm_ps, \
                 tc.tile_pool(name="gn_ps", bufs=2, space="PSUM") as gn_ps, \
                 tc.tile_pool(name="gn_sb", bufs=2) as gn_sb:
                t_warm = warm_ps.tile([128, 512], F32)
                # early HAM warm: PE activity from ~identb-arrival onward
                for r in range(64):
                    nc.tensor.matmul(out=t_warm[:, 0:128], lhsT=t_identb[:],
                                     rhs=t_identb[:], start=True, stop=True,
                                     skip_group_check=True)
                NDVE = 6
                sall = gn_sb.tile([128, CT], F32)
                sqall = gn_sb.tile([128, CT], F32)
                aggs = gn_sb.tile([128, CT, 2], F32)
                for t in range(CT):
                    if t < NDVE:
                        bst = gn_sb.tile([128, 2, 6], F32, tag="bst")
                        for c in range(2):
                            nc.vector.bn_stats(
                                out=bst[:, c, :],
                                in_=t_xb[:, t, 512 * c:512 * (c + 1)])
                        nc.vector.bn_aggr(out=aggs[:, t, :], in_=bst[:])
                    else:
                        scra = gn_sb.tile([128, L], BF16, tag="scra")
                        nc.scalar.activation(out=scra[:, 0:1024],
                                             in_=t_xb[:, t, 0:1024],
                                             func=AF.Square,
                                             accum_out=sqall[:, t:t + 1])
                        scrb = gn_sb.tile([128, L], BF16, tag="scrb")
                        nc.scalar.activation(out=scrb[:, 0:1024],
                                             in_=t_xb[:, t, 0:1024],
                                             func=AF.Identity,
                                             accum_out=sall[:, t:t + 1])
                    # keep PE ticking through the DMA phase (HAM warmup)
                    for r in range(2):
                        nc.tensor.matmul(
                            out=t_warm[:], lhsT=t_identb[:],
                            rhs=t_xb[:, t, r * 512:(r + 1) * 512],
                            start=True, stop=True, skip_group_check=True)
                v2sall = gn_sb.tile([128, 2 * CT], F32)
                nc.vector.tensor_scalar_mul(out=v2sall[:, NDVE:CT],
                                            in0=sall[:, NDVE:CT],
                                            scalar1=2.0 / L)
                nc.vector.tensor_scalar_mul(out=v2sall[:, CT + NDVE:],
                                            in0=sqall[:, NDVE:],
                                            scalar1=2.0 / L)
                nc.vector.tensor_copy(out=v2sall[:, 0:NDVE],
                                      in_=aggs[:, 0:NDVE, 0])
                nc.vector.tensor_mul(out=v2sall[:, CT:CT + NDVE],
                                     in0=aggs[:, 0:NDVE, 0],
                                     in1=aggs[:, 0:NDVE, 0])
                nc.vector.tensor_add(out=v2sall[:, CT:CT + NDVE],
                                     in0=v2sall[:, CT:CT + NDVE],
                                     in1=aggs[:, 0:NDVE, 1])
                p_g4 = gn_ps.tile([4, 2 * CT], F32)
                nc.tensor.matmul(out=p_g4[:], lhsT=t_ind, rhs=v2sall[:],
                                 start=True, stop=True)
                sc24 = gn_sb.tile([4, 2 * CT], F32)
                nc.vector.tensor_scalar_mul(out=sc24[:], in0=p_g4[:],
                                            scalar1=1.0 / 32.0)
                gs2 = gn_sb.tile([4, 2 * CT], F32)
                nc.vector.tensor_mul(out=gs2[:, 0:CT], in0=sc24[:, 0:CT],
                                     in1=sc24[:, 0:CT])
                nc.vector.tensor_sub(out=gs2[:, CT:], in0=sc24[:, CT:],
                                     in1=gs2[:, 0:CT])
                # rsqrt(var + eps) = exp(-0.5 * ln(var + eps)); Ln and Exp
                # share the natural_log_exp_and_others ACT table set
                nc.scalar.activation(out=gs2[:, CT:], in_=gs2[:, CT:],
                                     func=AF.Ln, bias=t_eps[0:4, :])
                nc.scalar.activation(out=gs2[:, CT:], in_=gs2[:, CT:],
                                     func=AF.Exp, scale=-0.5)
                nc.vector.tensor_copy(out=gs2[:, 0:CT], in_=sc24[:, 0:CT])
                p_c2 = gn_ps.tile([128, 2 * CT], F32)
                nc.tensor.matmul(out=p_c2[:], lhsT=t_indT[:], rhs=gs2[:],
                                 start=True, stop=True)
                svec = gn_sb.tile([128, CT], F32)
                nc.vector.tensor_mul(out=svec[:], in0=p_c2[:, CT:], in1=t_gnw)
                mub = small.tile([128, CT], BF16)
                nc.vector.tensor_mul(out=mub[:], in0=p_c2[:, 0:CT],
                                     in1=svec[:])
                for t in range(CT):
                    nc.vector.tensor_scalar_mul(
                        out=t_wqkvS[:, t, :], in0=t_wqkvT[:, t, :],
                        scalar1=svec[:, t:t + 1])

            # ---- qkv projection: fp8 DoubleRow over kt pairs, emitted per
            # 1024-col half of L; v transposed per 128-tile into AV weights.
            t_cb = small.tile([128, 3], F32)
            with tc.tile_pool(name="qkv_ps", bufs=2, space="PSUM") as qkv_ps, \
                 tc.tile_pool(name="vt_ps", bufs=2, space="PSUM") as vt_ps, \
                 tc.tile_pool(name="gn_ps2", bufs=1, space="PSUM") as gn_ps2:

                def emit_copy(p, ci, a):
                    sl = slice(1024 * a, 1024 * (a + 1))
                    if ci == 0:
                        nc.vector.tensor_scalar(
                            out=t_q2[:, sl], in0=p[:],
                            scalar1=t_cb[:, 0:1], scalar2=None, op0=ALU.add)
                    elif ci == 1:
                        nc.vector.tensor_scalar(
                            out=t_k2z0[0:64, sl], in0=p[0:64, :],
                            scalar1=t_cb[0:64, 1:2], scalar2=None, op0=ALU.add)
                        nc.vector.tensor_scalar(
                            out=t_k2z1[64:128, sl], in0=p[64:128, :],
                            scalar1=t_cb[64:128, 1:2], scalar2=None,
                            op0=ALU.add)
                    else:
                        nc.vector.tensor_scalar(
                            out=t_v2[:, sl], in0=p[:],
                            scalar1=t_cb[:, 2:3], scalar2=None, op0=ALU.add)

                pending = []
                for a in range(2):
                    for ci in ((0, 1, 2) if a == 0 else (2, 1, 0)):
                        p = qkv_ps.tile([128, 1024], F32, tag="qkv")
                        for h in range(2):
                            nn = 2 * a + h
                            for tp in range(4):
                                nc.tensor.matmul(
                                    out=p[:, 512 * h:512 * (h + 1)],
                                    lhsT=t_wqkvS[:, 2 * tp:2 * tp + 2,
                                                 ci * 128:(ci + 1) * 128],
                                    rhs=t_xb[:, 2 * tp:2 * tp + 2,
                                             nn * 512:(nn + 1) * 512],
                                    start=(tp == 0), stop=(tp == 3),
                                    perf_mode=DR)
                        if a == 0 and ci == 0:
                            # const[o] = sum_c W''[c, o] * mu_c
                            p_cn = gn_ps2.tile([128, 3], F32)
                            for cj in range(3):
                                for kt in range(CT):
                                    nc.tensor.matmul(
                                        out=p_cn[:, cj:cj + 1],
                                        lhsT=t_wqkvT[:, kt,
                                                     cj * 128:(cj + 1) * 128],
                                        rhs=mub[:, kt:kt + 1],
                                        start=(kt == 0), stop=(kt == CT - 1),
                                        skip_group_check=True)
                            nc.vector.tensor_sub(out=t_cb[:], in0=t_bvec,
                                                 in1=p_cn[:])
                        if a == 0 and ci < 2:
                            pending.append((p, ci, a))
                        else:
                            for args in pending:
                                emit_copy(*args)
                            pending = []
                            emit_copy(p, ci, a)
                        if ci == 2:
                            # transpose the 8 finished v tiles into AV weights
                            for sub in range(8):
                                i = a * 8 + sub
                                pt = vt_ps.tile([128, 128], BF16, tag="vt")
                                nc.tensor.transpose(
                                    out=pt[:],
                                    in_=t_v2[:, i * 128:(i + 1) * 128],
                                    identity=t_identb[:])
                                nc.vector.tensor_copy(out=t_vt[:, i, 0:64],
                                                      in_=pt[:, 0:64])
                                nc.vector.tensor_copy(out=t_vt[:, i, 192:256],
                                                      in_=pt[:, 64:128])

            # ---- attention in four query quarters over i-tile PAIRS.
            with tc.tile_pool(name="att_ps", bufs=3, space="PSUM") as att_ps, \
                 tc.tile_pool(name="av_ps", bufs=1, space="PSUM") as av_ps, \
                 tc.tile_pool(name="expp", bufs=8) as expp, \
                 tc.tile_pool(name="outp", bufs=3) as outp:

                cnt = [0]
                kz = (t_k2z0, t_k2z1)

                def regions(i, c0):
                    # single region per slab: 'a' (C31) when the (widened)
                    # strip deposit covers this quarter, else pure 'b' (C15)
                    hi = min(L, 128 * i + 192)
                    return [(0, 512, 0 if hi > c0 else 1)]

                def emit_exp(p, j, q, attp, es):
                    c0 = 512 * q
                    on_act = (cnt[0] * EACT8) % 8 < EACT8
                    cnt[0] += 1
                    regs = []
                    for s in (0, 1):
                        for (r0, r1, rg) in regions(2 * p + s, c0):
                            regs.append((s, r0, r1, rg))
                    # merge: if both slabs are a single full-range same region
                    if (len(regs) == 2 and regs[0][3] == regs[1][3]
                            and regs[0][1] == 0 and regs[0][2] == 512
                            and regs[1][1] == 0 and regs[1][2] == 512):
                        rg = regs[0][3]
                        if on_act:
                            nc.scalar.activation(
                                out=es[:, :, :], in_=attp[:, :, :],
                                func=AF.Exp,
                                bias=t_consts[:, 23 + 2 * rg + j:24 + 2 * rg + j])
                        else:
                            nc.vector.tensor_scalar(
                                out=es[:, :, :].bitcast(I8),
                                in0=attp[:, :, :],
                                scalar1=SCHR_A8,
                                scalar2=t_consts[:, 27 + 2 * rg + j:
                                                 28 + 2 * rg + j],
                                op0=ALU.mult, op1=ALU.add)
                        return
                    for (s, r0, r1, rg) in regs:
                        if on_act:
                            nc.scalar.activation(
                                out=es[:, s, r0:r1],
                                in_=attp[:, s, r0:r1],
                                func=AF.Exp,
                                bias=t_consts[:, 23 + 2 * rg + j:24 + 2 * rg + j])
                        else:
                            nc.vector.tensor_scalar(
                                out=es[:, s, r0:r1].bitcast(I8),
                                in0=attp[:, s, r0:r1],
                                scalar1=SCHR_A8,
                                scalar2=t_consts[:, 27 + 2 * rg + j:
                                                 28 + 2 * rg + j],
                                op0=ALU.mult, op1=ALU.add)

                def emit_av(p, ess, p_avs):
                    for j in (0, 1):
                        nc.tensor.matmul(
                            out=p_avs[j][:],
                            lhsT=t_vt[:, 2 * p:2 * p + 2,
                                      128 * j:128 * (j + 1)],
                            rhs=ess[j][:, :, :],
                            start=(p == 0), stop=(p == 7),
                            perf_mode=DR, skip_group_check=True)

                def att_quarter(q, projq, norm_thunk=None):
                    c0 = 512 * q
                    p_av0 = av_ps.tile([128, 512], F32, tag="av0")
                    p_av1 = av_ps.tile([128, 512], F32, tag="av1")
                    p_avs = (p_av0, p_av1)
                    pend = []
                    for p in range(8):
                        ess = []
                        for j in (0, 1):
                            attp = att_ps.tile([128, 2, 512], F32, tag="att")
                            for s in (0, 1):
                                i = 2 * p + s
                                lo = max(0, 128 * i - 64)
                                hi = min(L, 128 * i + 192)
                                aa = max(c0, lo)
                                bb = c0 + 512 if c0 < hi < c0 + 512 \
                                    else min(c0 + 512, hi)
                                dep = bb > aa
                                nc.tensor.matmul(
                                    out=attp[:, s, :],
                                    lhsT=kz[j][:, 128 * i:128 * (i + 1)],
                                    rhs=t_q2[:, c0:c0 + 512],
                                    start=True, stop=not dep,
                                    skip_group_check=True)
                                if dep:
                                    nc.tensor.matmul(
                                        out=attp[:, s, aa - c0:bb - c0],
                                        lhsT=t_identb[:],
                                        rhs=t_tb2[:, j, aa - (128 * i - 64):
                                                  bb - (128 * i - 64)],
                                        start=False, stop=True,
                                        skip_group_check=True)
                            es = expp.tile([128, 2, 512], F8, tag="exp")
                            emit_exp(p, j, q, attp, es)
                            ess.append(es)
                        pend.append((p, ess))
                        if norm_thunk and p == 1:
                            norm_thunk()
                            norm_thunk = None
                        if len(pend) > 1:
                            emit_av(*pend.pop(0), p_avs)
                        if projq and 2 <= p < 6:
                            projq.pop(0)()
                    for args in pend:
                        emit_av(*args, p_avs)
                    while projq:
                        projq.pop(0)()
                    return p_avs

                def emit_norm(q, p_avs):
                    c0 = 512 * q
                    t_bc = t_bcp[q % 2]
                    # den0 on row 64 of av0, den1 on row 32 of av1
                    nc.vector.tensor_copy(out=t_dn[64:65, 0:512],
                                          in_=p_avs[0][64:65, :])
                    nc.vector.tensor_copy(out=t_dn[32:33, 512:1024],
                                          in_=p_avs[1][32:33, :])
                    pb = att_ps.tile([128, 2, 512], F32, tag="att")
                    for u in (0, 1):
                        nc.tensor.matmul(out=pb[:, u, :],
                                         lhsT=t_selbc[:],
                                         rhs=t_dn[:, 512 * u:512 * (u + 1)],
                                         start=True, stop=True,
                                         skip_group_check=True)
                    # 1/den via single custom-DVE approx (18 bits, no ACT
                    # table switch, no 8x-slow iterative divide)
                    nc.vector.reciprocal_approx_fast(out=t_bc[:], in_=pb[:])
                    nc.vector.tensor_mul(out=t_outh[0:64, c0:c0 + 512],
                                         in0=p_avs[0][0:64, :],
                                         in1=t_bc[0:64, 0, :])
                    nc.vector.tensor_mul(out=t_outh[64:128, c0:c0 + 512],
                                         in0=p_avs[1][64:128, :],
                                         in1=t_bc[64:128, 1, :])

                def proj_thunks(q):
                    thunks = []
                    k = [0]
                    t0 = 512 * q
                    for mo in range(0, 8, 2):
                        def th(mo=mo):
                            p = att_ps.tile([128, 2, 512], F32, tag="att")
                            for u in (0, 1):
                                nc.tensor.matmul(
                                    out=p[:, u, :],
                                    lhsT=t_wproj2[:, (mo + u) * 128:
                                                  (mo + u + 1) * 128],
                                    rhs=t_outh[:, t0:t0 + 512],
                                    start=True, stop=True,
                                    skip_group_check=True)
                            po = outp.tile([128, 2, 512], BF16, tag="po")
                            if k[0] % 2 == 1:
                                nc.scalar.copy(out=po[:], in_=p[:])
                            else:
                                nc.vector.tensor_copy(out=po[:], in_=p[:])
                            k[0] += 1
                            nc.gpsimd.dma_start(
                                out=d_out[mo * 128:(mo + 2) * 128,
                                          t0:t0 + 512].rearrange(
                                              "(t p) l -> p t l", p=128),
                                in_=po[:])
                        thunks.append(th)
                    return thunks

                prev = None
                for q in range(4):
                    nt = None
                    if prev is not None:
                        nt = (lambda qq=q - 1, a=prev: emit_norm(qq, a))
                    pj = proj_thunks(q - 1) if q > 0 else None
                    prev = att_quarter(q, pj, nt)
                emit_norm(3, prev)
                for th in proj_thunks(3):
                    th()

    nc.compile()
    return nc


def _host_inputs(x, gn_w, gn_b, w_qkv, b_qkv, w_proj, b_proj, rel_bias):
    import ml_dtypes
    x2 = np.ascontiguousarray(x.reshape(C, L)).astype(np.float32)
    identb = np.eye(128).astype(ml_dtypes.bfloat16)
    ind = np.zeros((128, 4), dtype=np.float32)
    for p in range(128):
        ind[p, p // 32] = 1.0
    indT = np.ascontiguousarray(ind.T)
    gnw = np.ascontiguousarray(np.asarray(gn_w, np.float32).reshape(CT, 128).T)
    gnb = np.ascontiguousarray(np.asarray(gn_b, np.float32).reshape(CT, 128).T)

    w_qkv = np.asarray(w_qkv, np.float32)
    b_qkv = np.asarray(b_qkv, np.float32)
    w_proj = np.asarray(w_proj, np.float32)
    rel_bias = np.asarray(rel_bias, np.float32)

    # Toeplitz diag values D_h[u] = 8 * rel_bias[bucket(u - (L-1)), h]
    u = np.arange(2 * L - 1, dtype=np.int64)
    buckets = _bucket_np((u - (L - 1)).astype(np.int32))
    # strip table: tb2[p, m'] = D[p - m' + 2111] - C31
    p_idx = np.arange(128)[:, None]
    m_idx = np.arange(TBW2)[None, :]
    tb2_arg = p_idx - m_idx + (64 + L - 1)

    in_maps = []
    for d in range(N_CORES):
        heads = (2 * d, 2 * d + 1)
        wq, wk, wv, bq, bk, bv = [], [], [], [], [], []
        for h in heads:
            base = h * 3 * DH
            wq.append(w_qkv[base:base + DH] * SCALE)
            wk.append(w_qkv[base + DH:base + 2 * DH] * SCALE)
            wv.append(w_qkv[base + 2 * DH:base + 3 * DH])
            bq.append(b_qkv[base:base + DH] * SCALE)
            bk.append(b_qkv[base + DH:base + 2 * DH] * SCALE)
            bv.append(b_qkv[base + 2 * DH:base + 3 * DH])
        wall = np.concatenate(wq + wk + wv, axis=0)        # [384, 1024]
        wqkvT = np.ascontiguousarray(wall.T)               # [1024, 384]
        bvec = np.stack([np.concatenate(bq), np.concatenate(bk),
                         np.concatenate(bv)], axis=1)       # [128, 3]
        gnb_contrib = wall @ np.asarray(gn_b, np.float32)   # [384]
        bvec = bvec + gnb_contrib.reshape(3, 128).T
        # stacked proj weights: rows = [head0 dims, head1 dims]
        wproj2 = np.concatenate(
            [np.ascontiguousarray(w_proj[:, h * DH:(h + 1) * DH].T)
             for h in heads], axis=0)                       # [128, 1024]
        # per-head saturated-bias constants: C31 for q<k region (bucket 31),
        # C15 for q>k region (bucket 15); folded into the exp bias
        c31 = [8.0 * rel_bias[31, h] for h in heads]
        c15 = [8.0 * rel_bias[15, h] for h in heads]
        actb = np.tile(np.array([c31[0], c31[1], c15[0], c15[1]],
                                np.float32)[None, :], (128, 1))
        schb = SCHR_B8 + SCHR_A8 * actb
        tb2 = np.stack(
            [(8.0 * rel_bias[buckets, h])[tb2_arg] - 8.0 * rel_bias[31, h]
             for h in heads],
            axis=0).astype(ml_dtypes.bfloat16)              # [2, 128, 256]
        consts = np.concatenate([ind, gnw, gnb, bvec.astype(np.float32),
                                 actb, schb], axis=1).astype(np.float32)
        in_maps.append({
            "x": x2.astype(ml_dtypes.float8_e4m3fn),
            "wqkvT": wqkvT.astype(ml_dtypes.bfloat16),
            "consts": consts,
            "wproj2": wproj2.astype(ml_dtypes.bfloat16),
            "tb2": tb2, "identb": identb, "indT": indT,
        })
    return in_maps


def kernel(x, gn_w, gn_b, w_qkv, b_qkv, w_proj, b_proj, rel_bias, **run_kwargs):
    from concourse.bass_utils import run_bass_kernel_spmd
    if "nc" not in _CACHE:
        _CACHE["nc"] = _build_nc()
    nc = _CACHE["nc"]
    in_maps = _host_inputs(x, gn_w, gn_b, w_qkv, b_qkv, w_proj, b_proj, rel_bias)
    res = run_bass_kernel_spmd(nc, in_maps, core_ids=list(range(N_CORES)),
                               **run_kwargs)
    _CACHE["last_result"] = res
    acc = np.zeros((C, L), dtype=np.float32)
    for d in range(N_CORES):
        acc += np.asarray(res.results[d]["pout"], dtype=np.float32)
    out = acc + np.asarray(b_proj, np.float32)[:, None] \
        + np.asarray(x, np.float32).reshape(C, L)
    return out.reshape(B, C, L)
